# revision 1
# baseline (speedup 1.0000x reference)
"""Bayer kernel-prediction demosaic network on 8 Trainium2 NeuronCores.

Pipeline per core (one batch x one 96-row band of the 374-row quarter-res
kernel grid, with halo):
  - conv0 (4->64) via im2col (K=36) matmul, relu
  - conv1..3 (64->64) as 6 tap-matmuls each (3 paired taps K=128 using a
    column-shifted duplicate of the activations in partitions 64:128,
    plus 3 single taps K=64), relu
  - conv4 (64->490) same 6-matmul structure, 4 output-channel blocks,
    per-output-row tiles; exp(.) applied on PSUM eviction (ScalarE) with
    the conv bias folded into the activation bias -> E (bf16)
  - patch tensor Prep[tap_channel, pixel] gathered straight from the
    (padded) phase planes in DRAM with strided DMAs (bf16)
  - M = E * Prep on VectorE
  - numerator / denominator = group-mask matmuls (bf16) contracting the
    490 tap-channels -> [8, pix] in PSUM; out = num * reciprocal(den)
All conv-chain matmuls run in fp16 (full-rate, FWL weight loads) with
fp32 PSUM accumulation.
Host does phase extraction, weight re-layout, sharding w/ halo, and the
final pixel-shuffle assembly.
"""

import sys

sys.path.insert(0, "/opt/trn_rl_repo")

import numpy as np
import ml_dtypes

# ---------------- geometry constants ----------------
KS = 7
K2 = 49
BS = 2
H = W = 768
QH = QW = 384          # quarter-res
KR_TOT = 374           # valid kernel rows/cols
BANDS = 4              # bands per batch -> 8 cores
KR = 96                # kernel rows computed per core (94/92 valid)
RB = 16                # kernel rows per conv block
SRB = 8                # kernel rows per apply sub-block
NBLK = KR // RB
GW = 386               # conv grid width (L buffers, x36)
EW = 376               # apply/kernel grid width (374 valid + 2)
XW = 388               # x slab width (384 data + 4 zero)
XR = 106               # x slab rows (96 + 10)
ROWS0, ROWS1, ROWS2, ROWS3 = 24, 22, 20, 18   # conv0..conv3 out rows/block
F36 = ROWS0 * GW
F0, F1, F2, F3 = ROWS0 * GW, ROWS1 * GW, ROWS2 * GW, ROWS3 * GW
FE = SRB * EW          # flat apply pixels per sub-block
OUTF = KR * EW
MBLK = [0, 128, 256, 384, 490]     # channel block boundaries
# plane (x-slab channel) feeding each 49-tap chunk of the 490 kernels:
# x channels: 0=g0 1=b 2=r 3=g1 ; chunks: 3x red, 3x blue, (g0,g1)x2
CHUNK_PLANE = [2, 2, 2, 1, 1, 1, 0, 3, 0, 3]
# 49-chunk -> output group (greens pair up)
CHUNK_GROUP = [0, 1, 2, 3, 4, 5, 6, 6, 7, 7]

TRACE = False          # set True (module attr) to profile the run
LAST_EXEC_NS = None
LAST_RESULTS = None

_cache = {}


def _build():
    import concourse.bass as bass
    import concourse.bacc as bacc
    import concourse.mybir as mybir
    import concourse.tile as tile

    f32 = mybir.dt.float32
    f16 = mybir.dt.float16
    bf16 = mybir.dt.bfloat16
    AF = mybir.ActivationFunctionType
    ALU = mybir.AluOpType

    nc = bacc.Bacc("TRN2", target_bir_lowering=False, debug=False,
                   enable_asserts=False)

    xs = nc.dram_tensor("xs", [4, XR, XW], f16, kind="ExternalInput")
    xg = nc.dram_tensor("xg", [490, KR, EW], bf16, kind="ExternalInput")
    w0p = nc.dram_tensor("w0p", [36, 64], f16, kind="ExternalInput")
    wp = nc.dram_tensor("wp", [128, 9, 64], f16, kind="ExternalInput")
    ws = nc.dram_tensor("ws", [64, 9, 64], f16, kind="ExternalInput")
    w4p = nc.dram_tensor("w4p", [128, 3, 490], f16, kind="ExternalInput")
    w4s = nc.dram_tensor("w4s", [64, 3, 490], f16, kind="ExternalInput")
    b03 = nc.dram_tensor("b03", [64, 4], f32, kind="ExternalInput")
    b4 = nc.dram_tensor("b4", [128, 4], f32, kind="ExternalInput")
    gm = nc.dram_tensor("gm", [128, 4, 8], bf16, kind="ExternalInput")
    out = nc.dram_tensor("out", [8, OUTF], f32, kind="ExternalOutput")

    def ntiles(total, tsz=512):
        o = 0
        while o < total:
            n = min(tsz, total - o)
            yield o, n
            o += n

    with tile.TileContext(nc) as tc:
        with (
            tc.tile_pool(name="wts", bufs=1) as wts,
            tc.tile_pool(name="big", bufs=1) as big,
            tc.tile_pool(name="sm", bufs=2) as sm,
            tc.tile_pool(name="pscv", bufs=4, space="PSUM") as pscv,
            tc.tile_pool(name="psd", bufs=2, space="PSUM") as psd,
            tc.tile_pool(name="psn", bufs=2, space="PSUM") as psn,
        ):
            w0p_sb = wts.tile([36, 64], f16)
            wp_sb = wts.tile([128, 9, 64], f16)
            ws_sb = wts.tile([64, 9, 64], f16)
            w4p_sb = wts.tile([128, 3, 490], f16)
            w4s_sb = wts.tile([64, 3, 490], f16)
            b03_sb = wts.tile([64, 4], f32)
            b4_sb = wts.tile([128, 4], f32)
            gm_sb = wts.tile([128, 4, 8], bf16)
            for dst, src in ((w0p_sb, w0p), (wp_sb, wp), (ws_sb, ws),
                             (w4p_sb, w4p), (w4s_sb, w4s), (b03_sb, b03),
                             (b4_sb, b4), (gm_sb, gm)):
                nc.sync.dma_start(dst[:], src.ap())

            NS = RB // SRB

            def convtile(Lprev, ps, li, o, n):
                """conv1..3 tile: 6 tap matmuls as 3 rounds of 2 concurrent
                column tiles (left half {pair0, pair2, single1}, right half
                {pair1, single0, single2})."""
                mms = [
                    (0, wp_sb[:, 3 * li + 0, :], Lprev[0:128, o: o + n]),
                    (1, wp_sb[:, 3 * li + 1, :],
                     Lprev[0:128, o + GW: o + GW + n]),
                    (0, wp_sb[:, 3 * li + 2, :],
                     Lprev[0:128, o + 2 * GW: o + 2 * GW + n]),
                    (1, ws_sb[:, 3 * li + 0, :], Lprev[0:64, o + 2: o + 2 + n]),
                    (0, ws_sb[:, 3 * li + 1, :],
                     Lprev[0:64, o + GW + 2: o + GW + 2 + n]),
                    (1, ws_sb[:, 3 * li + 2, :],
                     Lprev[0:64, o + 2 * GW + 2: o + 2 * GW + 2 + n]),
                ]
                import os as _os
                if _os.environ.get("CONVTILE", "1") == "1":
                    seen = [False, False]
                    for idx, (h, lhsT, rhs) in enumerate(mms):
                        nc.tensor.matmul(ps[64 * h:64 * h + 64, 0:n], lhsT,
                                         rhs, start=not seen[h], stop=idx >= 4,
                                         tile_position=(0, 64 * h),
                                         skip_group_check=True)
                        seen[h] = True
                else:
                    for idx, (h, lhsT, rhs) in enumerate(mms):
                        nc.tensor.matmul(ps[0:64, 0:n], lhsT, rhs,
                                         start=idx == 0, stop=idx == 5)

            def ndpairs():
                """(o, n, o2, n2) chunk pairs covering [0, FE)."""
                import os as _os
                chunks = list(ntiles(FE))
                if _os.environ.get("NDPACK", "0") == "0":
                    for o, n in chunks:
                        yield o, n, None, 0
                    return
                for i in range(0, len(chunks), 2):
                    o, n = chunks[i]
                    o2, n2 = chunks[i + 1] if i + 1 < len(chunks) else (None, 0)
                    yield o, n, o2, n2

            def grp_mms(tag, blk, s, srcs, rec_s):
                """Column-tiled group reductions (den or num) + reciprocal or
                final multiply per chunk pair."""
                for o, n, o2, n2 in ndpairs():
                    nd = (psd if rec_s is not None else psn).tile(
                        [128, 512], f32, tag=tag, name=f"{tag}{blk}_{s}_{o}")
                    for m in range(4):
                        mm = MBLK[m + 1] - MBLK[m]
                        nc.tensor.matmul(nd[0:8, 0:n], gm_sb[0:mm, m, :],
                                         srcs[0:mm, m, o:o + n],
                                         start=(m == 0), stop=(m == 3),
                                         tile_position=(0, 0),
                                         skip_group_check=True)
                        if n2:
                            nc.tensor.matmul(nd[64:72, 0:n2],
                                             gm_sb[0:mm, m, :],
                                             srcs[0:mm, m, o2:o2 + n2],
                                             start=(m == 0), stop=(m == 3),
                                             tile_position=(0, 64),
                                             skip_group_check=True)
                    yield nd, o, n, o2, n2

            for blk in range(NBLK):
                R = blk * RB

                # ---- patch gather (prefetched: big DMA from DRAM) ----
                Preps = []
                for s in range(NS):
                    Prep = big.tile([128, 4, FE], bf16, tag="prep", bufs=2,
                                    name=f"Prep{blk}_{s}")
                    Preps.append(Prep)
                    for m in range(4):
                        mm = MBLK[m + 1] - MBLK[m]
                        src = bass.AP(
                            xg,
                            MBLK[m] * KR * EW + (R + s * SRB) * EW,
                            [[KR * EW, mm], [1, FE]],
                        )
                        nc.sync.dma_start(Prep[0:mm, m, :], src)

                # ---- conv0 input: im2col ----
                x36 = big.tile([36, F36], f16, tag="x36", bufs=1,
                               name=f"x36{blk}")
                for dy in range(3):
                    for dx in range(3):
                        p = (3 * dy + dx) * 4
                        src = bass.AP(
                            xs,
                            (R + dy) * XW + dx,
                            [[XR * XW, 4], [XW, ROWS0], [1, GW]],
                        )
                        nc.sync.dma_start(x36[p:p + 4, :], src)

                # ---- conv0 (per-tile shifted-duplicate copies follow) ----
                L0 = big.tile([128, F0 + 2], f16, tag="la", bufs=1,
                              name=f"L0{blk}")
                prev = 0
                for o, n in ntiles(F0):
                    ps = pscv.tile([128, 512], f32, tag="pscv",
                                   name=f"ps0_{blk}_{o}")
                    nc.tensor.matmul(ps[0:64, 0:n], w0p_sb[:, :],
                                     x36[:, o:o + n], start=True, stop=True)
                    nc.scalar.activation(L0[0:64, o:o + n], ps[0:64, 0:n],
                                         AF.Relu, bias=b03_sb[:, 0:1])
                    if o > 0:
                        nc.sync.dma_start(L0[64:128, prev:o],
                                            L0[0:64, prev + 1:o + 1])
                        prev = o
                nc.sync.dma_start(L0[64:128, prev:F0 - 1],
                                    L0[0:64, prev + 1:F0])

                # ---- conv1..conv3 ----
                Lprev = L0
                for li, (Fi, tag) in enumerate(((F1, "lb"), (F2, "la"),
                                                (F3, "lb"))):
                    Li = big.tile([128, Fi + 2], f16, tag=tag, bufs=1,
                                  name=f"L{li + 1}{blk}")
                    prev = 0
                    for o, n in ntiles(Fi):
                        ps = pscv.tile([128, 512], f32, tag="pscv",
                                       name=f"ps{li + 1}_{blk}_{o}")
                        convtile(Lprev, ps, li, o, n)
                        mrg = sm.tile([64, 512], f32, tag="mrg",
                                      name=f"mrg{li}_{blk}_{o}")
                        # ACT evacuates the right psum half (bias folded in),
                        # DVE adds the left half, ACT applies relu.
                        nc.scalar.activation(mrg[0:64, 0:n], ps[64:128, 0:n],
                                             AF.Identity,
                                             bias=b03_sb[:, li + 1:li + 2])
                        nc.vector.tensor_add(mrg[0:64, 0:n], ps[0:64, 0:n],
                                             mrg[0:64, 0:n])
                        nc.scalar.activation(Li[0:64, o:o + n], mrg[0:64, 0:n],
                                             AF.Relu)
                        if o > 0:
                            nc.sync.dma_start(Li[64:128, prev:o],
                                                Li[0:64, prev + 1:o + 1])
                            prev = o
                    nc.sync.dma_start(Li[64:128, prev:Fi - 1],
                                        Li[0:64, prev + 1:Fi])
                    Lprev = Li

                # ---- conv4 + exp -> E (bf16), sub-block at a time ----
                def conv4_sub(s):
                    E = big.tile([128, 4, FE], bf16, tag=f"e{s}", bufs=1,
                                 name=f"E{blk}_{s}")
                    for m in range(4):
                        mm = MBLK[m + 1] - MBLK[m]
                        for r in range(SRB):
                            rho = s * SRB + r
                            ps4 = pscv.tile([128, 512], f32, tag="pscv",
                                            name=f"ps4_{blk}_{rho}_{m}")
                            for dy in range(3):
                                nc.tensor.matmul(
                                    ps4[0:mm, 0:EW],
                                    w4p_sb[:, dy, MBLK[m]:MBLK[m + 1]],
                                    Lprev[0:128, (rho + dy) * GW:
                                          (rho + dy) * GW + EW],
                                    start=(dy == 0), stop=False)
                            for dy in range(3):
                                nc.tensor.matmul(
                                    ps4[0:mm, 0:EW],
                                    w4s_sb[:, dy, MBLK[m]:MBLK[m + 1]],
                                    Lprev[0:64, (rho + dy) * GW + 2:
                                          (rho + dy) * GW + 2 + EW],
                                    start=False, stop=(dy == 2))
                            nc.scalar.activation(
                                E[0:mm, m, r * EW:(r + 1) * EW],
                                ps4[0:mm, 0:EW], AF.Exp,
                                bias=b4_sb[0:mm, m:m + 1])
                    return E

                def den_sub(blk, s, E):
                    rec = sm.tile([8, FE], f32, tag="rec", bufs=2,
                                  name=f"rec{blk}_{s}")
                    for nd, o, n, o2, n2 in grp_mms("den", blk, s, E, rec):
                        nc.vector.reciprocal_approx_fast(rec[0:8, o:o + n],
                                                         nd[0:8, 0:n])
                        if n2:
                            nc.vector.reciprocal_approx_fast(
                                rec[0:8, o2:o2 + n2], nd[64:72, 0:n2])
                    return rec

                def mult_sub(s, E):
                    # E *= Prep in place, split across VectorE and GpSimdE
                    import os as _os
                    h = FE * 3 // 5
                    use_gp = _os.environ.get("GPMUL", "1") == "1"
                    for m in range(4):
                        mm = MBLK[m + 1] - MBLK[m]
                        if use_gp:
                            nc.vector.tensor_mul(E[0:mm, m, 0:h],
                                                 E[0:mm, m, 0:h],
                                                 Preps[s][0:mm, m, 0:h])
                            nc.gpsimd.tensor_mul(E[0:mm, m, h:FE],
                                                 E[0:mm, m, h:FE],
                                                 Preps[s][0:mm, m, h:FE])
                        else:
                            nc.vector.tensor_mul(E[0:mm, m, :],
                                                 E[0:mm, m, :],
                                                 Preps[s][0:mm, m, :])

                def num_sub(blk, s, E, rec):
                    for nd, o, n, o2, n2 in grp_mms("num", blk, s, E, None):
                        res = sm.tile([8, 512], f32, tag="res", bufs=3,
                                      name=f"res{blk}_{s}_{o}")
                        nc.vector.tensor_mul(res[0:8, 0:n], nd[0:8, 0:n],
                                             rec[0:8, o:o + n])
                        nc.sync.dma_start(
                            out.ap()[0:8, (R + s * SRB) * EW + o:
                                     (R + s * SRB) * EW + o + n],
                            res[0:8, 0:n])
                        if n2:
                            res2 = sm.tile([8, 512], f32, tag="res", bufs=3,
                                           name=f"res{blk}_{s}_{o2}")
                            nc.vector.tensor_mul(res2[0:8, 0:n2],
                                                 nd[64:72, 0:n2],
                                                 rec[0:8, o2:o2 + n2])
                            nc.sync.dma_start(
                                out.ap()[0:8, (R + s * SRB) * EW + o2:
                                         (R + s * SRB) * EW + o2 + n2],
                                res2[0:8, 0:n2])

                E0 = conv4_sub(0)
                rec0 = den_sub(blk, 0, E0)
                E1 = conv4_sub(1)
                mult_sub(0, E0)
                rec1 = den_sub(blk, 1, E1)
                num_sub(blk, 0, E0, rec0)
                mult_sub(1, E1)
                num_sub(blk, 1, E1, rec1)

    nc.compile()
    return nc


def _host_prep(inputs):
    mosaic = np.asarray(inputs["mosaic"], dtype=np.float32)
    gray = mosaic.sum(axis=1)                       # [2, 768, 768]
    g0 = gray[:, 0::2, 0::2]
    b_ = gray[:, 1::2, 0::2]
    r = gray[:, 0::2, 1::2]
    g1 = gray[:, 1::2, 1::2]
    x4 = np.stack([g0, b_, r, g1], axis=1)          # [2, 4, 384, 384]
    xpad = np.zeros((BS, 4, QH + 4, XW), dtype=np.float32)
    xpad[:, :, :QH, :QW] = x4

    W0 = np.asarray(inputs["W0"], np.float32)
    w0p = np.ascontiguousarray(W0.transpose(2, 3, 1, 0).reshape(36, 64))

    wp = np.empty((128, 9, 64), np.float32)
    ws = np.empty((64, 9, 64), np.float32)
    for li, wname in enumerate(("W1", "W2", "W3")):
        Wi = np.asarray(inputs[wname], np.float32)   # [64, 64, 3, 3]
        wp[0:64, 3 * li:3 * li + 3, :] = Wi[:, :, :, 0].transpose(1, 2, 0)
        wp[64:128, 3 * li:3 * li + 3, :] = Wi[:, :, :, 1].transpose(1, 2, 0)
        ws[:, 3 * li:3 * li + 3, :] = Wi[:, :, :, 2].transpose(1, 2, 0)

    W4 = np.asarray(inputs["W4"], np.float32)        # [490, 64, 3, 3]
    w4p = np.empty((128, 3, 490), np.float32)
    w4s = np.empty((64, 3, 490), np.float32)
    w4p[0:64] = W4[:, :, :, 0].transpose(1, 2, 0)
    w4p[64:128] = W4[:, :, :, 1].transpose(1, 2, 0)
    w4s[:] = W4[:, :, :, 2].transpose(1, 2, 0)

    b03 = np.stack([np.asarray(inputs[f"b{i}"], np.float32)
                    for i in range(4)], axis=1)      # [64, 4]
    b4v = np.asarray(inputs["b4"], np.float32)
    b4p = np.zeros((128, 4), np.float32)
    for c in range(490):
        b4p[c % 128, c // 128] = b4v[c]

    gmk = np.zeros((128, 4, 8), ml_dtypes.bfloat16)
    for c in range(490):
        gmk[c % 128, c // 128, CHUNK_GROUP[c // 49]] = 1

    xpad_bf = xpad.astype(ml_dtypes.bfloat16)
    w0p16 = w0p.astype(np.float16)
    wp16 = wp.astype(np.float16)
    ws16 = ws.astype(np.float16)
    w4p16 = w4p.astype(np.float16)
    w4s16 = w4s.astype(np.float16)
    in_maps = []
    for b in range(BS):
        for band in range(BANDS):
            r0 = band * 94
            slab = np.zeros((4, XR, XW), np.float16)
            hi = min(QH, r0 + XR)
            slab[:, 0:hi - r0, :] = xpad[b, :, r0:hi, :].astype(np.float16)
            # shifted-plane (im2col) tensor for the kernel-apply patches:
            # xg[49*j + 7*dy + dx, jr, jc] = plane_j[r0 + jr + 2 + dy, jc + 2 + dx]
            xgp = np.empty((490, KR, EW), ml_dtypes.bfloat16)
            for j in range(10):
                pl = xpad_bf[b, CHUNK_PLANE[j]]
                for dy in range(KS):
                    for dx in range(KS):
                        c = 49 * j + 7 * dy + dx
                        xgp[c] = pl[r0 + 2 + dy: r0 + 2 + dy + KR,
                                    2 + dx: 2 + dx + EW]
            in_maps.append({
                "xs": slab, "xg": xgp,
                "w0p": w0p16, "wp": wp16, "ws": ws16,
                "w4p": w4p16, "w4s": w4s16,
                "b03": b03, "b4": b4p, "gm": gmk,
            })
    aux = {"g0": g0, "b_": b_, "r": r, "g1": g1}
    return in_maps, aux


def _assemble(results, aux):
    full = np.empty((BS, 3, 2 * KR_TOT, 2 * KR_TOT), np.float32)
    # quarter-res computed planes [8, 374, 374] per batch
    for b in range(BS):
        qs = []
        for band in range(BANDS):
            core = b * BANDS + band
            o = results[core]["out"].reshape(8, KR, EW)
            nvalid = min(94, KR_TOT - band * 94)
            qs.append(o[:, :nvalid, :KR_TOT])
        q = np.concatenate(qs, axis=1)               # [8, 374, 374]
        crop = (slice(5, 5 + KR_TOT), slice(5, 5 + KR_TOT))
        r_pass = aux["r"][b][crop]
        b_pass = aux["b_"][b][crop]
        g0_pass = aux["g0"][b][crop]
        g1_pass = aux["g1"][b][crop]
        # red
        full[b, 0, 0::2, 0::2] = q[0]
        full[b, 0, 0::2, 1::2] = r_pass
        full[b, 0, 1::2, 0::2] = q[1]
        full[b, 0, 1::2, 1::2] = q[2]
        # green
        full[b, 1, 0::2, 0::2] = g0_pass
        full[b, 1, 0::2, 1::2] = q[6]
        full[b, 1, 1::2, 0::2] = q[7]
        full[b, 1, 1::2, 1::2] = g1_pass
        # blue
        full[b, 2, 0::2, 0::2] = q[3]
        full[b, 2, 0::2, 1::2] = q[4]
        full[b, 2, 1::2, 0::2] = b_pass
        full[b, 2, 1::2, 1::2] = q[5]
    return full


def kernel(**inputs):
    global LAST_EXEC_NS, LAST_RESULTS
    from concourse.bass_utils import run_bass_kernel_spmd

    if "nc" not in _cache:
        _cache["nc"] = _build()
    nc = _cache["nc"]

    in_maps, aux = _host_prep(inputs)
    kw = {}
    if TRACE:
        kw["trace"] = True
    res = run_bass_kernel_spmd(nc, in_maps, core_ids=list(range(8)), **kw)
    LAST_EXEC_NS = res.exec_time_ns
    LAST_RESULTS = res
    return _assemble(res.results, aux)



# revision 2
# speedup vs baseline: 2.1475x; 2.1475x over previous
"""Bayer kernel-prediction demosaic network on 8 Trainium2 NeuronCores.

v2: row-interleaved activation layout. Each conv layer's output lives as
[128 partitions, row-pairs * GW]: partitions 0:64 hold channel c of EVEN
rows, 64:128 hold channel c of ODD rows of each row-pair. Consequences:
  - a 3x3 conv = 12 matmuls per 512-px tile (3 K=128 dy-pairs + 3 K=64
    singles per row parity), two concurrent 64-col streams via column
    groups, accumulating into one [128, 512] PSUM tile
  - eviction is ONE scalar-engine relu+bias ACT per tile (no PSUM-half
    merge, no shifted-duplicate DMA copies)
  - conv4 (64->490) runs per (row, m-block of 128): 5 matmuls per row
    (3 K=128 dy-pairs + 1 K=128 column-shift pair from a dup buffer +
    1 K=64 single); 2-row PSUM batches evicted by one exp ACT -> E
  - den/num group reductions 4-way column-packed at tile positions
    (0,0)/(0,32)/(0,64)/(0,96) -> packed reciprocal / final multiply
  - emission is software-pipelined across the 6 row blocks so TensorE
    always has queued work (keeps the PE HAM clock gate warm)
Host does phase extraction, weight re-layout, patch-tensor build,
sharding w/ halo, and the final pixel-shuffle assembly.
"""

import sys

sys.path.insert(0, "/opt/trn_rl_repo")

import numpy as np
import ml_dtypes

# ---------------- geometry constants ----------------
KS = 7
K2 = 49
BS = 2
H = W = 768
QH = QW = 384          # quarter-res
KR_TOT = 374           # valid kernel rows/cols
BANDS = 4              # bands per batch -> 8 cores
KR = 96                # kernel rows computed per core (94/92 valid)
RB = 16                # kernel rows per block
SRB = 8                # kernel rows per apply sub-block
NBLK = KR // RB
GW = 386               # conv grid width
EW = 376               # apply/kernel grid width (374 valid + 2)
XW = 388               # x slab width (384 data + 4 zero)
XR = 106               # x slab rows (96 + 10)
NP0, NP1, NP2, NP3 = 12, 11, 10, 9     # row-pairs per conv output/block
F0, F1, F2, F3 = NP0 * GW, NP1 * GW, NP2 * GW, NP3 * GW
FE = SRB * EW          # flat apply pixels per sub-block (3008)
OUTF = KR * EW
MBLK = [0, 128, 256, 384, 490]     # channel block boundaries
# plane (x-slab channel) feeding each 49-tap chunk of the 490 kernels:
# x channels: 0=g0 1=b 2=r 3=g1 ; chunks: 3x red, 3x blue, (g0,g1)x2
CHUNK_PLANE = [2, 2, 2, 1, 1, 1, 0, 3, 0, 3]
# 49-chunk -> output group (greens pair up)
CHUNK_GROUP = [0, 1, 2, 3, 4, 5, 6, 6, 7, 7]
# den/num column chunks, grouped into packed PSUM tiles
CHUNKS = [(o, min(512, FE - o)) for o in range(0, FE, 512)]
CGROUPS = [CHUNKS[0:4], CHUNKS[4:6]]
MULT_H = 2280          # DVE/GpSimd split point for the E*Prep multiply

TRACE = False          # set True (module attr) to profile the run
LAST_EXEC_NS = None
LAST_RESULTS = None

_cache = {}


def _build():
    import concourse.bass as bass
    import concourse.bacc as bacc
    import concourse.mybir as mybir
    import concourse.tile as tile

    f32 = mybir.dt.float32
    f16 = mybir.dt.float16
    bf16 = mybir.dt.bfloat16
    AF = mybir.ActivationFunctionType

    nc = bacc.Bacc("TRN2", target_bir_lowering=False, debug=False,
                   enable_asserts=False)

    xs = nc.dram_tensor("xs", [4, XR, XW], f16, kind="ExternalInput")
    xg = nc.dram_tensor("xg", [490, KR, EW], bf16, kind="ExternalInput")
    w0 = nc.dram_tensor("w0", [100, 64], f16, kind="ExternalInput")
    wpe = nc.dram_tensor("wpe", [128, 9, 64], f16, kind="ExternalInput")
    wpo = nc.dram_tensor("wpo", [128, 9, 64], f16, kind="ExternalInput")
    wsng = nc.dram_tensor("wsng", [128, 9, 64], f16, kind="ExternalInput")
    w4pe = nc.dram_tensor("w4pe", [128, 3, 490], f16, kind="ExternalInput")
    w4po = nc.dram_tensor("w4po", [128, 3, 490], f16, kind="ExternalInput")
    w4d = nc.dram_tensor("w4d", [128, 2, 490], f16, kind="ExternalInput")
    w4s = nc.dram_tensor("w4s", [128, 490], f16, kind="ExternalInput")
    b03 = nc.dram_tensor("b03", [128, 4], f32, kind="ExternalInput")
    b4 = nc.dram_tensor("b4", [128, 4], f32, kind="ExternalInput")
    gm = nc.dram_tensor("gm", [128, 4, 8], bf16, kind="ExternalInput")
    out = nc.dram_tensor("out", [8, OUTF], f32, kind="ExternalOutput")

    def ntiles(total, tsz=512):
        o = 0
        while o < total:
            n = min(tsz, total - o)
            yield o, n
            o += n

    with tile.TileContext(nc) as tc:
        with (
            tc.tile_pool(name="wts", bufs=1) as wts,
            tc.tile_pool(name="xp", bufs=1) as xp,
            tc.tile_pool(name="lp", bufs=1) as lp,
            tc.tile_pool(name="pp", bufs=2) as pp,
            tc.tile_pool(name="ep", bufs=1) as ep,
            tc.tile_pool(name="smp", bufs=2) as smp,
            tc.tile_pool(name="pscv", bufs=2, space="PSUM") as pscv,
            tc.tile_pool(name="ps4p", bufs=2, space="PSUM") as ps4p,
            tc.tile_pool(name="psa", bufs=2, space="PSUM") as psa,
        ):
            w0_sb = wts.tile([100, 64], f16)
            wpe_sb = wts.tile([128, 9, 64], f16)
            wpo_sb = wts.tile([128, 9, 64], f16)
            wsng_sb = wts.tile([128, 9, 64], f16)
            w4pe_sb = wts.tile([128, 3, 490], f16)
            w4po_sb = wts.tile([128, 3, 490], f16)
            w4d_sb = wts.tile([128, 2, 490], f16)
            w4s_sb = wts.tile([128, 490], f16)
            b03_sb = wts.tile([128, 4], f32)
            b4_sb = wts.tile([128, 4], f32)
            gm_sb = wts.tile([128, 4, 8], bf16)
            for dst, src in ((w0_sb, w0), (wpe_sb, wpe), (wpo_sb, wpo),
                             (wsng_sb, wsng), (w4pe_sb, w4pe),
                             (w4po_sb, w4po), (w4d_sb, w4d), (w4s_sb, w4s),
                             (b03_sb, b03), (b4_sb, b4), (gm_sb, gm)):
                nc.sync.dma_start(dst[:], src.ap())

            st = {}

            def emit_prefetch(b):
                R = b * RB
                x36t = xp.tile([100, F0], f16, tag="x36", bufs=1,
                               name=f"x36_{b}")
                for dy in range(3):
                    for dx in range(3):
                        p = 4 * (3 * dy + dx)
                        dims = [[XR * XW, 4], [2 * XW, NP0], [1, GW]]
                        nc.sync.dma_start(
                            x36t[p:p + 4, :],
                            bass.AP(xs, (R + dy) * XW + dx, dims))
                        nc.sync.dma_start(
                            x36t[64 + p:64 + p + 4, :],
                            bass.AP(xs, (R + 1 + dy) * XW + dx, dims))
                preps = []
                for s in range(2):
                    P = pp.tile([128, 4, FE], bf16, tag="prep", bufs=2,
                                name=f"prep{b}_{s}")
                    for m in range(4):
                        mm = MBLK[m + 1] - MBLK[m]
                        src = bass.AP(
                            xg,
                            MBLK[m] * KR * EW + (R + s * SRB) * EW,
                            [[KR * EW, mm], [1, FE]])
                        nc.sync.dma_start(P[0:mm, m, :], src)
                    preps.append(P)
                st[('x36', b)] = x36t
                st[('prep', b)] = preps

            def emit_conv0(b):
                x36t = st.pop(('x36', b))
                L0 = lp.tile([128, F0 + 2], f16, tag="l0", bufs=1,
                             name=f"L0_{b}")
                for o, n in ntiles(F0):
                    ps = pscv.tile([128, 512], f32, tag="pscv",
                                   name=f"ps0_{b}_{o}")
                    nc.tensor.matmul(ps[0:64, 0:n], w0_sb[0:36, :],
                                     x36t[0:36, o:o + n], start=True,
                                     stop=True, skip_group_check=True)
                    nc.tensor.matmul(ps[64:128, 0:n], w0_sb[64:100, :],
                                     x36t[64:100, o:o + n], start=True,
                                     stop=True, skip_group_check=True)
                    nc.scalar.activation(L0[0:128, o:o + n], ps[0:128, 0:n],
                                         AF.Relu, bias=b03_sb[:, 0:1])
                st[('L', b)] = L0

            def emit_convi(b, li, Fi, tg, nb):
                Lp = st[('L', b)]
                Li = lp.tile([128, Fi + 2], f16, tag=tg, bufs=nb,
                             name=f"L{li + 1}_{b}")
                for o, n in ntiles(Fi):
                    ps = pscv.tile([128, 512], f32, tag="pscv",
                                   name=f"ps{li + 1}_{b}_{o}")
                    for dx in range(3):
                        nc.tensor.matmul(ps[0:64, 0:n],
                                         wpe_sb[:, 3 * li + dx, :],
                                         Lp[0:128, o + dx:o + dx + n],
                                         start=(dx == 0), stop=False,
                                         skip_group_check=True)
                        nc.tensor.matmul(
                            ps[64:128, 0:n], wpo_sb[:, 3 * li + dx, :],
                            Lp[0:128, o + GW + dx:o + GW + dx + n],
                            start=(dx == 0), stop=False,
                            skip_group_check=True)
                    for dx in range(3):
                        nc.tensor.matmul(
                            ps[0:64, 0:n], wsng_sb[0:64, 3 * li + dx, :],
                            Lp[0:64, o + GW + dx:o + GW + dx + n],
                            start=False, stop=(dx == 2),
                            skip_group_check=True)
                        nc.tensor.matmul(
                            ps[64:128, 0:n], wsng_sb[64:128, 3 * li + dx, :],
                            Lp[64:128, o + dx:o + dx + n],
                            start=False, stop=(dx == 2),
                            skip_group_check=True)
                    nc.scalar.activation(Li[0:128, o:o + n], ps[0:128, 0:n],
                                         AF.Relu,
                                         bias=b03_sb[:, li + 1:li + 2])
                st[('L', b)] = Li

            def emit_ldup(b):
                L3 = st[('L', b)]
                de = lp.tile([128, F3 + 2], f16, tag="ldupe", bufs=1,
                             name=f"de_{b}")
                do = lp.tile([128, F3 + 2], f16, tag="ldupo", bufs=1,
                             name=f"do_{b}")
                HS = 5 * GW
                for c0, c1 in ((0, HS), (HS, F3)):
                    nc.sync.dma_start(de[0:64, c0:c1], L3[0:64, c0 + 1:c1 + 1])
                    nc.sync.dma_start(de[64:128, c0:c1],
                                      L3[0:64, c0 + 2:c1 + 2])
                    nc.sync.dma_start(do[0:64, c0:c1],
                                      L3[64:128, c0 + 1:c1 + 1])
                    nc.sync.dma_start(do[64:128, c0:c1],
                                      L3[64:128, c0 + 2:c1 + 2])
                st[('ldup', b)] = (de, do)

            def conv4_sub(b, s):
                L3 = st[('L', b)]
                de, do = st[('ldup', b)]
                E = ep.tile([128, 4, FE], bf16, tag=f"e{s}", bufs=1,
                            name=f"E{b}_{s}")
                for rp in range(4):
                    rho = s * 4 + rp
                    oe = rho * GW
                    oo = (rho + 1) * GW
                    for m in range(4):
                        mm = MBLK[m + 1] - MBLK[m]
                        ms = slice(MBLK[m], MBLK[m + 1])
                        ps4 = ps4p.tile([128, 2, 512], f32, tag="ps4",
                                        name=f"ps4_{b}_{rho}_{m}")
                        for dx in range(3):
                            nc.tensor.matmul(
                                ps4[0:mm, 0, 0:EW], w4pe_sb[:, dx, ms],
                                L3[0:128, oe + dx:oe + dx + EW],
                                start=(dx == 0), stop=False,
                                skip_group_check=True)
                            nc.tensor.matmul(
                                ps4[0:mm, 1, 0:EW], w4po_sb[:, dx, ms],
                                L3[0:128, oo + dx:oo + dx + EW],
                                start=(dx == 0), stop=False,
                                skip_group_check=True)
                        nc.tensor.matmul(ps4[0:mm, 0, 0:EW],
                                         w4d_sb[:, 0, ms],
                                         de[0:128, oo:oo + EW],
                                         start=False, stop=False,
                                         skip_group_check=True)
                        nc.tensor.matmul(ps4[0:mm, 1, 0:EW],
                                         w4d_sb[:, 1, ms],
                                         do[0:128, oe:oe + EW],
                                         start=False, stop=False,
                                         skip_group_check=True)
                        nc.tensor.matmul(ps4[0:mm, 0, 0:EW],
                                         w4s_sb[0:64, ms],
                                         L3[0:64, oo:oo + EW],
                                         start=False, stop=True,
                                         skip_group_check=True)
                        nc.tensor.matmul(ps4[0:mm, 1, 0:EW],
                                         w4s_sb[64:128, ms],
                                         L3[64:128, oe:oe + EW],
                                         start=False, stop=True,
                                         skip_group_check=True)
                        nc.scalar.activation(
                            E[0:mm, m, 2 * rp * EW:(2 * rp + 2) * EW],
                            ps4[0:mm, :, 0:EW], AF.Exp,
                            bias=b4_sb[0:mm, m:m + 1])
                return E

            def grp_tile(tag, b, s, ti, grp, E):
                nd = psa.tile([128, 512], f32, tag="psa",
                              name=f"{tag}{b}_{s}_{ti}")
                for m in range(4):
                    mm = MBLK[m + 1] - MBLK[m]
                    for j, (o, n) in enumerate(grp):
                        nc.tensor.matmul(nd[32 * j:32 * j + 8, 0:n],
                                         gm_sb[0:mm, m, :],
                                         E[0:mm, m, o:o + n],
                                         start=(m == 0), stop=(m == 3),
                                         tile_position=(0, 32 * j),
                                         skip_group_check=True)
                return nd

            def den_sub(b, s, E):
                recs = []
                for ti, grp in enumerate(CGROUPS):
                    nd = grp_tile("den", b, s, ti, grp, E)
                    Pn = 32 * (len(grp) - 1) + 8
                    rec = smp.tile([128, 512], f32, tag="rec", bufs=4,
                                   name=f"rec{b}_{s}_{ti}")
                    nc.vector.reciprocal_approx_fast(rec[0:Pn, :],
                                                     nd[0:Pn, 0:512])
                    recs.append(rec)
                return recs

            def mult_sub(b, s, E):
                Prep = st[('prep', b)][s]
                for m in range(4):
                    mm = MBLK[m + 1] - MBLK[m]
                    nc.vector.tensor_mul(E[0:mm, m, 0:MULT_H],
                                         E[0:mm, m, 0:MULT_H],
                                         Prep[0:mm, m, 0:MULT_H])
                    nc.gpsimd.tensor_mul(E[0:mm, m, MULT_H:FE],
                                         E[0:mm, m, MULT_H:FE],
                                         Prep[0:mm, m, MULT_H:FE])

            def num_sub(b, s, E, recs):
                base = (b * RB + s * SRB) * EW
                for ti, grp in enumerate(CGROUPS):
                    nd = grp_tile("num", b, s, ti, grp, E)
                    Pn = 32 * (len(grp) - 1) + 8
                    res = smp.tile([128, 512], f32, tag="res", bufs=2,
                                   name=f"res{b}_{s}_{ti}")
                    nc.vector.tensor_mul(res[0:Pn, :], nd[0:Pn, 0:512],
                                         recs[ti][0:Pn, :])
                    for j, (o, n) in enumerate(grp):
                        nc.sync.dma_start(
                            out.ap()[0:8, base + o:base + o + n],
                            res[32 * j:32 * j + 8, 0:n])

            def emit_chain(b):
                emit_conv0(b)
                emit_convi(b, 0, F1, "l1", 1)
                emit_convi(b, 1, F2, "l2", 1)
                emit_convi(b, 2, F3, "l3", 2)

            emit_prefetch(0)
            emit_chain(0)
            for b in range(NBLK):
                if b + 1 < NBLK:
                    emit_prefetch(b + 1)
                emit_ldup(b)
                E0 = conv4_sub(b, 0)
                recs0 = den_sub(b, 0, E0)
                E1 = conv4_sub(b, 1)
                mult_sub(b, 0, E0)
                recs1 = den_sub(b, 1, E1)
                if b + 1 < NBLK:
                    emit_conv0(b + 1)
                num_sub(b, 0, E0, recs0)
                mult_sub(b, 1, E1)
                if b + 1 < NBLK:
                    emit_convi(b + 1, 0, F1, "l1", 1)
                num_sub(b, 1, E1, recs1)
                if b + 1 < NBLK:
                    emit_convi(b + 1, 1, F2, "l2", 1)
                    emit_convi(b + 1, 2, F3, "l3", 2)

    nc.compile()
    return nc


def _host_prep(inputs):
    mosaic = np.asarray(inputs["mosaic"], dtype=np.float32)
    gray = mosaic.sum(axis=1)                       # [2, 768, 768]
    g0 = gray[:, 0::2, 0::2]
    b_ = gray[:, 1::2, 0::2]
    r = gray[:, 0::2, 1::2]
    g1 = gray[:, 1::2, 1::2]
    x4 = np.stack([g0, b_, r, g1], axis=1)          # [2, 4, 384, 384]
    xpad = np.zeros((BS, 4, QH + 4, XW), dtype=np.float32)
    xpad[:, :, :QH, :QW] = x4

    W0 = np.asarray(inputs["W0"], np.float32)
    w0v = np.zeros((100, 64), np.float32)
    w0flat = W0.transpose(2, 3, 1, 0).reshape(36, 64)
    w0v[0:36] = w0flat
    w0v[64:100] = w0flat

    wpe = np.zeros((128, 9, 64), np.float32)
    wpo = np.zeros((128, 9, 64), np.float32)
    wsng = np.zeros((128, 9, 64), np.float32)
    for li, wname in enumerate(("W1", "W2", "W3")):
        Wi = np.asarray(inputs[wname], np.float32)   # [64out, 64in, 3, 3]
        for dx in range(3):
            wpe[0:64, 3 * li + dx, :] = Wi[:, :, 0, dx].T
            wpe[64:128, 3 * li + dx, :] = Wi[:, :, 1, dx].T
            wpo[0:64, 3 * li + dx, :] = Wi[:, :, 1, dx].T
            wpo[64:128, 3 * li + dx, :] = Wi[:, :, 2, dx].T
            wsng[0:64, 3 * li + dx, :] = Wi[:, :, 2, dx].T
            wsng[64:128, 3 * li + dx, :] = Wi[:, :, 0, dx].T

    W4 = np.asarray(inputs["W4"], np.float32)        # [490, 64, 3, 3]
    w4pe = np.zeros((128, 3, 490), np.float32)
    w4po = np.zeros((128, 3, 490), np.float32)
    w4d = np.zeros((128, 2, 490), np.float32)
    w4s = np.zeros((128, 490), np.float32)
    for dx in range(3):
        w4pe[0:64, dx, :] = W4[:, :, 0, dx].T
        w4pe[64:128, dx, :] = W4[:, :, 1, dx].T
        w4po[0:64, dx, :] = W4[:, :, 1, dx].T
        w4po[64:128, dx, :] = W4[:, :, 2, dx].T
    w4d[0:64, 0, :] = W4[:, :, 2, 1].T
    w4d[64:128, 0, :] = W4[:, :, 2, 2].T
    w4d[0:64, 1, :] = W4[:, :, 0, 1].T
    w4d[64:128, 1, :] = W4[:, :, 0, 2].T
    w4s[0:64, :] = W4[:, :, 2, 0].T
    w4s[64:128, :] = W4[:, :, 0, 0].T

    b03 = np.zeros((128, 4), np.float32)
    for i in range(4):
        bi = np.asarray(inputs[f"b{i}"], np.float32)
        b03[0:64, i] = bi
        b03[64:128, i] = bi
    b4v = np.asarray(inputs["b4"], np.float32)
    b4p = np.zeros((128, 4), np.float32)
    for c in range(490):
        b4p[c % 128, c // 128] = b4v[c]

    gmk = np.zeros((128, 4, 8), ml_dtypes.bfloat16)
    for c in range(490):
        gmk[c % 128, c // 128, CHUNK_GROUP[c // 49]] = 1

    xpad_bf = xpad.astype(ml_dtypes.bfloat16)
    wcast = {
        "w0": w0v.astype(np.float16),
        "wpe": wpe.astype(np.float16),
        "wpo": wpo.astype(np.float16),
        "wsng": wsng.astype(np.float16),
        "w4pe": w4pe.astype(np.float16),
        "w4po": w4po.astype(np.float16),
        "w4d": w4d.astype(np.float16),
        "w4s": w4s.astype(np.float16),
        "b03": b03, "b4": b4p, "gm": gmk,
    }
    in_maps = []
    for b in range(BS):
        for band in range(BANDS):
            r0 = band * 94
            slab = np.zeros((4, XR, XW), np.float16)
            hi = min(QH, r0 + XR)
            slab[:, 0:hi - r0, :] = xpad[b, :, r0:hi, :].astype(np.float16)
            # shifted-plane (im2col) tensor for the kernel-apply patches:
            # xg[49*j + 7*dy + dx, jr, jc] = plane_j[r0 + jr + 2 + dy, jc + 2 + dx]
            xgp = np.empty((490, KR, EW), ml_dtypes.bfloat16)
            for j in range(10):
                pl = xpad_bf[b, CHUNK_PLANE[j]]
                for dy in range(KS):
                    for dx in range(KS):
                        c = 49 * j + 7 * dy + dx
                        xgp[c] = pl[r0 + 2 + dy: r0 + 2 + dy + KR,
                                    2 + dx: 2 + dx + EW]
            im = {"xs": slab, "xg": xgp}
            im.update(wcast)
            in_maps.append(im)
    aux = {"g0": g0, "b_": b_, "r": r, "g1": g1}
    return in_maps, aux


def _assemble(results, aux):
    full = np.empty((BS, 3, 2 * KR_TOT, 2 * KR_TOT), np.float32)
    # quarter-res computed planes [8, 374, 374] per batch
    for b in range(BS):
        qs = []
        for band in range(BANDS):
            core = b * BANDS + band
            o = results[core]["out"].reshape(8, KR, EW)
            nvalid = min(94, KR_TOT - band * 94)
            qs.append(o[:, :nvalid, :KR_TOT])
        q = np.concatenate(qs, axis=1)               # [8, 374, 374]
        crop = (slice(5, 5 + KR_TOT), slice(5, 5 + KR_TOT))
        r_pass = aux["r"][b][crop]
        b_pass = aux["b_"][b][crop]
        g0_pass = aux["g0"][b][crop]
        g1_pass = aux["g1"][b][crop]
        # red
        full[b, 0, 0::2, 0::2] = q[0]
        full[b, 0, 0::2, 1::2] = r_pass
        full[b, 0, 1::2, 0::2] = q[1]
        full[b, 0, 1::2, 1::2] = q[2]
        # green
        full[b, 1, 0::2, 0::2] = g0_pass
        full[b, 1, 0::2, 1::2] = q[6]
        full[b, 1, 1::2, 0::2] = q[7]
        full[b, 1, 1::2, 1::2] = g1_pass
        # blue
        full[b, 2, 0::2, 0::2] = q[3]
        full[b, 2, 0::2, 1::2] = q[4]
        full[b, 2, 1::2, 0::2] = b_pass
        full[b, 2, 1::2, 1::2] = q[5]
    return full


def kernel(**inputs):
    global LAST_EXEC_NS, LAST_RESULTS
    from concourse.bass_utils import run_bass_kernel_spmd

    if "nc" not in _cache:
        _cache["nc"] = _build()
    nc = _cache["nc"]

    in_maps, aux = _host_prep(inputs)
    kw = {}
    if TRACE:
        kw["trace"] = True
    res = run_bass_kernel_spmd(nc, in_maps, core_ids=list(range(8)), **kw)
    LAST_EXEC_NS = res.exec_time_ns
    LAST_RESULTS = res
    return _assemble(res.results, aux)


# revision 6
# speedup vs baseline: 2.1969x; 1.0230x over previous
"""Bayer kernel-prediction demosaic network on 8 Trainium2 NeuronCores.

v2: row-interleaved activation layout. Each conv layer's output lives as
[128 partitions, row-pairs * GW]: partitions 0:64 hold channel c of EVEN
rows, 64:128 hold channel c of ODD rows of each row-pair. Consequences:
  - a 3x3 conv = 12 matmuls per 512-px tile (3 K=128 dy-pairs + 3 K=64
    singles per row parity), two concurrent 64-col streams via column
    groups, accumulating into one [128, 512] PSUM tile
  - eviction is ONE scalar-engine relu+bias ACT per tile (no PSUM-half
    merge, no shifted-duplicate DMA copies)
  - conv4 (64->490) runs per (row, m-block of 128): 5 matmuls per row
    (3 K=128 dy-pairs + 1 K=128 column-shift pair from a dup buffer +
    1 K=64 single); 2-row PSUM batches evicted by one exp ACT -> E
  - den/num group reductions 4-way column-packed at tile positions
    (0,0)/(0,32)/(0,64)/(0,96) -> packed reciprocal / final multiply
  - emission is software-pipelined across the 6 row blocks so TensorE
    always has queued work (keeps the PE HAM clock gate warm)
Host does phase extraction, weight re-layout, patch-tensor build,
sharding w/ halo, and the final pixel-shuffle assembly.
"""

import sys

sys.path.insert(0, "/opt/trn_rl_repo")

import numpy as np
import ml_dtypes

# ---------------- geometry constants ----------------
KS = 7
K2 = 49
BS = 2
H = W = 768
QH = QW = 384          # quarter-res
KR_TOT = 374           # valid kernel rows/cols
BANDS = 4              # bands per batch -> 8 cores
KR = 96                # kernel rows computed per core (94/92 valid)
RB = 16                # kernel rows per block
SRB = 8                # kernel rows per apply sub-block
NBLK = KR // RB
GW = 386               # conv grid width
EW = 376               # apply/kernel grid width (374 valid + 2)
XW = 388               # x slab width (384 data + 4 zero)
XR = 106               # x slab rows (96 + 10)
NP0, NP1, NP2, NP3 = 12, 11, 10, 9     # row-pairs per conv output/block
F0, F1, F2, F3 = NP0 * GW, NP1 * GW, NP2 * GW, NP3 * GW
FE = SRB * EW          # flat apply pixels per sub-block (3008)
OUTF = KR * EW
MBLK = [0, 128, 256, 384, 490]     # channel block boundaries
# plane (x-slab channel) feeding each 49-tap chunk of the 490 kernels:
# x channels: 0=g0 1=b 2=r 3=g1 ; chunks: 3x red, 3x blue, (g0,g1)x2
CHUNK_PLANE = [2, 2, 2, 1, 1, 1, 0, 3, 0, 3]
# 49-chunk -> output group (greens pair up)
CHUNK_GROUP = [0, 1, 2, 3, 4, 5, 6, 6, 7, 7]
# den/num column chunks, grouped into packed PSUM tiles
CHUNKS = [(o, min(512, FE - o)) for o in range(0, FE, 512)]
CGROUPS = [CHUNKS[0:4], CHUNKS[4:6]]
MULT_H = 2280          # DVE/GpSimd split point for the E*Prep multiply

TRACE = False          # set True (module attr) to profile the run
LAST_EXEC_NS = None
LAST_RESULTS = None

_cache = {}


def _build():
    import concourse.bass as bass
    import concourse.bacc as bacc
    import concourse.mybir as mybir
    import concourse.tile as tile

    f32 = mybir.dt.float32
    f16 = mybir.dt.float16
    bf16 = mybir.dt.bfloat16
    AF = mybir.ActivationFunctionType

    nc = bacc.Bacc("TRN2", target_bir_lowering=False, debug=False,
                   enable_asserts=False)

    xs = nc.dram_tensor("xs", [4, XR, XW], f16, kind="ExternalInput")
    xg = nc.dram_tensor("xg", [490, KR, EW], bf16, kind="ExternalInput")
    w0 = nc.dram_tensor("w0", [100, 64], f16, kind="ExternalInput")
    wpe = nc.dram_tensor("wpe", [128, 9, 64], f16, kind="ExternalInput")
    wpo = nc.dram_tensor("wpo", [128, 9, 64], f16, kind="ExternalInput")
    wsng = nc.dram_tensor("wsng", [128, 9, 64], f16, kind="ExternalInput")
    w4pe = nc.dram_tensor("w4pe", [128, 3, 490], f16, kind="ExternalInput")
    w4po = nc.dram_tensor("w4po", [128, 3, 490], f16, kind="ExternalInput")
    w4d = nc.dram_tensor("w4d", [128, 2, 490], f16, kind="ExternalInput")
    w4s = nc.dram_tensor("w4s", [128, 490], f16, kind="ExternalInput")
    b03 = nc.dram_tensor("b03", [128, 4], f32, kind="ExternalInput")
    b4 = nc.dram_tensor("b4", [128, 4], f32, kind="ExternalInput")
    gm = nc.dram_tensor("gm", [128, 4, 8], bf16, kind="ExternalInput")
    out = nc.dram_tensor("out", [8, OUTF], f32, kind="ExternalOutput")

    def ntiles(total, tsz=512):
        o = 0
        while o < total:
            n = min(tsz, total - o)
            yield o, n
            o += n

    with tile.TileContext(nc) as tc:
        with (
            tc.tile_pool(name="wts", bufs=1) as wts,
            tc.tile_pool(name="xp", bufs=1) as xp,
            tc.tile_pool(name="lp", bufs=1) as lp,
            tc.tile_pool(name="pp", bufs=2) as pp,
            tc.tile_pool(name="ep", bufs=1) as ep,
            tc.tile_pool(name="smp", bufs=2) as smp,
            tc.tile_pool(name="pscv", bufs=2, space="PSUM") as pscv,
            tc.tile_pool(name="ps4p", bufs=2, space="PSUM") as ps4p,
            tc.tile_pool(name="psa", bufs=2, space="PSUM") as psa,
        ):
            w0_sb = wts.tile([100, 64], f16)
            wpe_sb = wts.tile([128, 9, 64], f16)
            wpo_sb = wts.tile([128, 9, 64], f16)
            wsng_sb = wts.tile([128, 9, 64], f16)
            w4pe_sb = wts.tile([128, 3, 490], f16)
            w4po_sb = wts.tile([128, 3, 490], f16)
            w4d_sb = wts.tile([128, 2, 490], f16)
            w4s_sb = wts.tile([128, 490], f16)
            b03_sb = wts.tile([128, 4], f32)
            b4_sb = wts.tile([128, 4], f32)
            gm_sb = wts.tile([128, 4, 8], bf16)
            for dst, src in ((w0_sb, w0), (wpe_sb, wpe), (wpo_sb, wpo),
                             (wsng_sb, wsng), (w4pe_sb, w4pe),
                             (w4po_sb, w4po), (w4d_sb, w4d), (w4s_sb, w4s),
                             (b03_sb, b03), (b4_sb, b4), (gm_sb, gm)):
                nc.sync.dma_start(dst[:], src.ap())

            st = {}

            def emit_x36(b):
                R = b * RB
                x36t = xp.tile([100, F0], f16, tag="x36", bufs=1,
                               name=f"x36_{b}")
                for dy in range(3):
                    for dx in range(3):
                        p = 4 * (3 * dy + dx)
                        dims = [[XR * XW, 4], [2 * XW, NP0], [1, GW]]
                        nc.sync.dma_start(
                            x36t[p:p + 4, :],
                            bass.AP(xs, (R + dy) * XW + dx, dims))
                        nc.sync.dma_start(
                            x36t[64 + p:64 + p + 4, :],
                            bass.AP(xs, (R + 1 + dy) * XW + dx, dims))
                st[('x36', b)] = x36t

            def emit_prep(b):
                R = b * RB
                preps = []
                HF = FE // 2
                for s in range(2):
                    P = pp.tile([128, 4, FE], bf16, tag="prep", bufs=2,
                                name=f"prep{b}_{s}")
                    for m in range(4):
                        mm = MBLK[m + 1] - MBLK[m]
                        for fo in (0, HF):
                            src = bass.AP(
                                xg,
                                MBLK[m] * KR * EW + (R + s * SRB) * EW + fo,
                                [[KR * EW, mm], [1, HF]])
                            nc.sync.dma_start(P[0:mm, m, fo:fo + HF], src)
                    preps.append(P)
                st[('prep', b)] = preps

            def emit_conv0(b):
                x36t = st.pop(('x36', b))
                L0 = lp.tile([128, F0 + 2], f16, tag="l0", bufs=1,
                             name=f"L0_{b}")
                for o, n in ntiles(F0):
                    ps = pscv.tile([128, 512], f32, tag="pscv",
                                   name=f"ps0_{b}_{o}")
                    nc.tensor.matmul(ps[0:64, 0:n], w0_sb[0:36, :],
                                     x36t[0:36, o:o + n], start=True,
                                     stop=True, skip_group_check=True)
                    nc.tensor.matmul(ps[64:128, 0:n], w0_sb[64:100, :],
                                     x36t[64:100, o:o + n], start=True,
                                     stop=True, skip_group_check=True)
                    nc.scalar.activation(L0[0:128, o:o + n], ps[0:128, 0:n],
                                         AF.Relu, bias=b03_sb[:, 0:1])
                st[('L', b)] = L0

            def emit_convi(b, li, Fi, tg, nb):
                Lp = st[('L', b)]
                Li = lp.tile([128, Fi + 2], f16, tag=tg, bufs=nb,
                             name=f"L{li + 1}_{b}")
                for o, n in ntiles(Fi):
                    ps = pscv.tile([128, 512], f32, tag="pscv",
                                   name=f"ps{li + 1}_{b}_{o}")
                    for dx in range(3):
                        nc.tensor.matmul(ps[0:64, 0:n],
                                         wpe_sb[:, 3 * li + dx, :],
                                         Lp[0:128, o + dx:o + dx + n],
                                         start=(dx == 0), stop=False,
                                         skip_group_check=True)
                        nc.tensor.matmul(
                            ps[64:128, 0:n], wpo_sb[:, 3 * li + dx, :],
                            Lp[0:128, o + GW + dx:o + GW + dx + n],
                            start=(dx == 0), stop=False,
                            skip_group_check=True)
                    for dx in range(3):
                        nc.tensor.matmul(
                            ps[0:64, 0:n], wsng_sb[0:64, 3 * li + dx, :],
                            Lp[0:64, o + GW + dx:o + GW + dx + n],
                            start=False, stop=(dx == 2),
                            skip_group_check=True)
                        nc.tensor.matmul(
                            ps[64:128, 0:n], wsng_sb[64:128, 3 * li + dx, :],
                            Lp[64:128, o + dx:o + dx + n],
                            start=False, stop=(dx == 2),
                            skip_group_check=True)
                    nc.scalar.activation(Li[0:128, o:o + n], ps[0:128, 0:n],
                                         AF.Relu,
                                         bias=b03_sb[:, li + 1:li + 2])
                st[('L', b)] = Li

            def emit_ldup(b):
                L3 = st[('L', b)]
                de = lp.tile([128, F3 + 2], f16, tag="ldupe", bufs=1,
                             name=f"de_{b}")
                do = lp.tile([128, F3 + 2], f16, tag="ldupo", bufs=1,
                             name=f"do_{b}")
                for c0, c1 in ((0, 2 * GW + 378), (2 * GW + 378, 5 * GW),
                               (5 * GW, F3)):
                    nc.sync.dma_start(de[0:64, c0:c1], L3[0:64, c0 + 1:c1 + 1])
                    nc.sync.dma_start(de[64:128, c0:c1],
                                      L3[0:64, c0 + 2:c1 + 2])
                    nc.sync.dma_start(do[0:64, c0:c1],
                                      L3[64:128, c0 + 1:c1 + 1])
                    nc.sync.dma_start(do[64:128, c0:c1],
                                      L3[64:128, c0 + 2:c1 + 2])
                st[('ldup', b)] = (de, do)

            def conv4_sub(b, s):
                L3 = st[('L', b)]
                de, do = st[('ldup', b)]
                E = ep.tile([128, 4, FE], bf16, tag=f"e{s}", bufs=1,
                            name=f"E{b}_{s}")
                for rp in range(4):
                    rho = s * 4 + rp
                    oe = rho * GW
                    oo = (rho + 1) * GW
                    for m in range(4):
                        mm = MBLK[m + 1] - MBLK[m]
                        ms = slice(MBLK[m], MBLK[m + 1])
                        ps4 = ps4p.tile([128, 2, 512], f32, tag="ps4",
                                        name=f"ps4_{b}_{rho}_{m}")
                        for dx in range(3):
                            nc.tensor.matmul(
                                ps4[0:mm, 0, 0:EW], w4pe_sb[:, dx, ms],
                                L3[0:128, oe + dx:oe + dx + EW],
                                start=(dx == 0), stop=False,
                                skip_group_check=True)
                            nc.tensor.matmul(
                                ps4[0:mm, 1, 0:EW], w4po_sb[:, dx, ms],
                                L3[0:128, oo + dx:oo + dx + EW],
                                start=(dx == 0), stop=False,
                                skip_group_check=True)
                        nc.tensor.matmul(ps4[0:mm, 0, 0:EW],
                                         w4s_sb[0:64, ms],
                                         L3[0:64, oo:oo + EW],
                                         start=False, stop=False,
                                         skip_group_check=True)
                        nc.tensor.matmul(ps4[0:mm, 1, 0:EW],
                                         w4s_sb[64:128, ms],
                                         L3[64:128, oe:oe + EW],
                                         start=False, stop=False,
                                         skip_group_check=True)
                        nc.tensor.matmul(ps4[0:mm, 0, 0:EW],
                                         w4d_sb[:, 0, ms],
                                         de[0:128, oo:oo + EW],
                                         start=False, stop=True,
                                         skip_group_check=True)
                        nc.tensor.matmul(ps4[0:mm, 1, 0:EW],
                                         w4d_sb[:, 1, ms],
                                         do[0:128, oe:oe + EW],
                                         start=False, stop=True,
                                         skip_group_check=True)
                        nc.scalar.activation(
                            E[0:mm, m, 2 * rp * EW:(2 * rp + 2) * EW],
                            ps4[0:mm, :, 0:EW], AF.Exp,
                            bias=b4_sb[0:mm, m:m + 1])
                return E

            def grp_tile(tag, b, s, ti, grp, E):
                nd = psa.tile([128, 512], f32, tag="psa",
                              name=f"{tag}{b}_{s}_{ti}")
                for m in range(4):
                    mm = MBLK[m + 1] - MBLK[m]
                    for j, (o, n) in enumerate(grp):
                        nc.tensor.matmul(nd[32 * j:32 * j + 8, 0:n],
                                         gm_sb[0:mm, m, :],
                                         E[0:mm, m, o:o + n],
                                         start=(m == 0), stop=(m == 3),
                                         tile_position=(0, 32 * j),
                                         skip_group_check=True)
                return nd

            def den_sub(b, s, E):
                recs = []
                for ti, grp in enumerate(CGROUPS):
                    nd = grp_tile("den", b, s, ti, grp, E)
                    Pn = 32 * (len(grp) - 1) + 8
                    rec = smp.tile([128, 512], f32, tag="rec", bufs=4,
                                   name=f"rec{b}_{s}_{ti}")
                    nc.vector.reciprocal_approx_fast(rec[0:Pn, :],
                                                     nd[0:Pn, 0:512])
                    recs.append(rec)
                return recs

            def mult_sub(b, s, E):
                Prep = st[('prep', b)][s]
                for m in range(4):
                    mm = MBLK[m + 1] - MBLK[m]
                    nc.vector.tensor_mul(E[0:mm, m, 0:MULT_H],
                                         E[0:mm, m, 0:MULT_H],
                                         Prep[0:mm, m, 0:MULT_H])
                    nc.gpsimd.tensor_mul(E[0:mm, m, MULT_H:FE],
                                         E[0:mm, m, MULT_H:FE],
                                         Prep[0:mm, m, MULT_H:FE])

            def num_sub(b, s, E, recs):
                base = (b * RB + s * SRB) * EW
                for ti, grp in enumerate(CGROUPS):
                    nd = grp_tile("num", b, s, ti, grp, E)
                    Pn = 32 * (len(grp) - 1) + 8
                    res = smp.tile([128, 512], f32, tag="res", bufs=2,
                                   name=f"res{b}_{s}_{ti}")
                    nc.vector.tensor_mul(res[0:Pn, :], nd[0:Pn, 0:512],
                                         recs[ti][0:Pn, :])
                    for j, (o, n) in enumerate(grp):
                        nc.sync.dma_start(
                            out.ap()[0:8, base + o:base + o + n],
                            res[32 * j:32 * j + 8, 0:n])

            emit_x36(0)
            emit_conv0(0)
            emit_prep(0)
            emit_convi(0, 0, F1, "l1", 1)
            emit_convi(0, 1, F2, "l2", 1)
            emit_convi(0, 2, F3, "l3", 2)
            for b in range(NBLK):
                emit_ldup(b)
                if b + 1 < NBLK:
                    emit_x36(b + 1)
                    emit_prep(b + 1)
                E0 = conv4_sub(b, 0)
                recs0 = den_sub(b, 0, E0)
                E1 = conv4_sub(b, 1)
                mult_sub(b, 0, E0)
                recs1 = den_sub(b, 1, E1)
                if b + 1 < NBLK:
                    emit_conv0(b + 1)
                num_sub(b, 0, E0, recs0)
                mult_sub(b, 1, E1)
                if b + 1 < NBLK:
                    emit_convi(b + 1, 0, F1, "l1", 1)
                num_sub(b, 1, E1, recs1)
                if b + 1 < NBLK:
                    emit_convi(b + 1, 1, F2, "l2", 1)
                    emit_convi(b + 1, 2, F3, "l3", 2)

    nc.compile()
    return nc


def _host_prep(inputs):
    mosaic = np.asarray(inputs["mosaic"], dtype=np.float32)
    gray = mosaic.sum(axis=1)                       # [2, 768, 768]
    g0 = gray[:, 0::2, 0::2]
    b_ = gray[:, 1::2, 0::2]
    r = gray[:, 0::2, 1::2]
    g1 = gray[:, 1::2, 1::2]
    x4 = np.stack([g0, b_, r, g1], axis=1)          # [2, 4, 384, 384]
    xpad = np.zeros((BS, 4, QH + 4, XW), dtype=np.float32)
    xpad[:, :, :QH, :QW] = x4

    W0 = np.asarray(inputs["W0"], np.float32)
    w0v = np.zeros((100, 64), np.float32)
    w0flat = W0.transpose(2, 3, 1, 0).reshape(36, 64)
    w0v[0:36] = w0flat
    w0v[64:100] = w0flat

    wpe = np.zeros((128, 9, 64), np.float32)
    wpo = np.zeros((128, 9, 64), np.float32)
    wsng = np.zeros((128, 9, 64), np.float32)
    for li, wname in enumerate(("W1", "W2", "W3")):
        Wi = np.asarray(inputs[wname], np.float32)   # [64out, 64in, 3, 3]
        for dx in range(3):
            wpe[0:64, 3 * li + dx, :] = Wi[:, :, 0, dx].T
            wpe[64:128, 3 * li + dx, :] = Wi[:, :, 1, dx].T
            wpo[0:64, 3 * li + dx, :] = Wi[:, :, 1, dx].T
            wpo[64:128, 3 * li + dx, :] = Wi[:, :, 2, dx].T
            wsng[0:64, 3 * li + dx, :] = Wi[:, :, 2, dx].T
            wsng[64:128, 3 * li + dx, :] = Wi[:, :, 0, dx].T

    W4 = np.asarray(inputs["W4"], np.float32)        # [490, 64, 3, 3]
    w4pe = np.zeros((128, 3, 490), np.float32)
    w4po = np.zeros((128, 3, 490), np.float32)
    w4d = np.zeros((128, 2, 490), np.float32)
    w4s = np.zeros((128, 490), np.float32)
    for dx in range(3):
        w4pe[0:64, dx, :] = W4[:, :, 0, dx].T
        w4pe[64:128, dx, :] = W4[:, :, 1, dx].T
        w4po[0:64, dx, :] = W4[:, :, 1, dx].T
        w4po[64:128, dx, :] = W4[:, :, 2, dx].T
    w4d[0:64, 0, :] = W4[:, :, 2, 1].T
    w4d[64:128, 0, :] = W4[:, :, 2, 2].T
    w4d[0:64, 1, :] = W4[:, :, 0, 1].T
    w4d[64:128, 1, :] = W4[:, :, 0, 2].T
    w4s[0:64, :] = W4[:, :, 2, 0].T
    w4s[64:128, :] = W4[:, :, 0, 0].T

    b03 = np.zeros((128, 4), np.float32)
    for i in range(4):
        bi = np.asarray(inputs[f"b{i}"], np.float32)
        b03[0:64, i] = bi
        b03[64:128, i] = bi
    b4v = np.asarray(inputs["b4"], np.float32)
    b4p = np.zeros((128, 4), np.float32)
    for c in range(490):
        b4p[c % 128, c // 128] = b4v[c]

    gmk = np.zeros((128, 4, 8), ml_dtypes.bfloat16)
    for c in range(490):
        gmk[c % 128, c // 128, CHUNK_GROUP[c // 49]] = 1

    xpad_bf = xpad.astype(ml_dtypes.bfloat16)
    wcast = {
        "w0": w0v.astype(np.float16),
        "wpe": wpe.astype(np.float16),
        "wpo": wpo.astype(np.float16),
        "wsng": wsng.astype(np.float16),
        "w4pe": w4pe.astype(np.float16),
        "w4po": w4po.astype(np.float16),
        "w4d": w4d.astype(np.float16),
        "w4s": w4s.astype(np.float16),
        "b03": b03, "b4": b4p, "gm": gmk,
    }
    in_maps = []
    for b in range(BS):
        for band in range(BANDS):
            r0 = band * 94
            slab = np.zeros((4, XR, XW), np.float16)
            hi = min(QH, r0 + XR)
            slab[:, 0:hi - r0, :] = xpad[b, :, r0:hi, :].astype(np.float16)
            # shifted-plane (im2col) tensor for the kernel-apply patches:
            # xg[49*j + 7*dy + dx, jr, jc] = plane_j[r0 + jr + 2 + dy, jc + 2 + dx]
            xgp = np.empty((490, KR, EW), ml_dtypes.bfloat16)
            for j in range(10):
                pl = xpad_bf[b, CHUNK_PLANE[j]]
                for dy in range(KS):
                    for dx in range(KS):
                        c = 49 * j + 7 * dy + dx
                        xgp[c] = pl[r0 + 2 + dy: r0 + 2 + dy + KR,
                                    2 + dx: 2 + dx + EW]
            im = {"xs": slab, "xg": xgp}
            im.update(wcast)
            in_maps.append(im)
    aux = {"g0": g0, "b_": b_, "r": r, "g1": g1}
    return in_maps, aux


def _assemble(results, aux):
    full = np.empty((BS, 3, 2 * KR_TOT, 2 * KR_TOT), np.float32)
    # quarter-res computed planes [8, 374, 374] per batch
    for b in range(BS):
        qs = []
        for band in range(BANDS):
            core = b * BANDS + band
            o = results[core]["out"].reshape(8, KR, EW)
            nvalid = min(94, KR_TOT - band * 94)
            qs.append(o[:, :nvalid, :KR_TOT])
        q = np.concatenate(qs, axis=1)               # [8, 374, 374]
        crop = (slice(5, 5 + KR_TOT), slice(5, 5 + KR_TOT))
        r_pass = aux["r"][b][crop]
        b_pass = aux["b_"][b][crop]
        g0_pass = aux["g0"][b][crop]
        g1_pass = aux["g1"][b][crop]
        # red
        full[b, 0, 0::2, 0::2] = q[0]
        full[b, 0, 0::2, 1::2] = r_pass
        full[b, 0, 1::2, 0::2] = q[1]
        full[b, 0, 1::2, 1::2] = q[2]
        # green
        full[b, 1, 0::2, 0::2] = g0_pass
        full[b, 1, 0::2, 1::2] = q[6]
        full[b, 1, 1::2, 0::2] = q[7]
        full[b, 1, 1::2, 1::2] = g1_pass
        # blue
        full[b, 2, 0::2, 0::2] = q[3]
        full[b, 2, 0::2, 1::2] = q[4]
        full[b, 2, 1::2, 0::2] = b_pass
        full[b, 2, 1::2, 1::2] = q[5]
    return full


def kernel(**inputs):
    global LAST_EXEC_NS, LAST_RESULTS
    from concourse.bass_utils import run_bass_kernel_spmd

    if "nc" not in _cache:
        _cache["nc"] = _build()
    nc = _cache["nc"]

    in_maps, aux = _host_prep(inputs)
    kw = {}
    if TRACE:
        kw["trace"] = True
    res = run_bass_kernel_spmd(nc, in_maps, core_ids=list(range(8)), **kw)
    LAST_EXEC_NS = res.exec_time_ns
    LAST_RESULTS = res
    return _assemble(res.results, aux)
